# revision 7
# baseline (speedup 1.0000x reference)
"""BitLinear forward on 8 Trainium2 NeuronCores.

out = (x_q @ w_q) * (beta * gamma)
  a      = mean(weight);  w_q = sign(weight - a)
  gamma  = max|x| per row; x_q = clip(x/(gamma+eps), -(1-eps), 1-eps)
  beta   = max|weight|

Sharding: data-parallel over rows of x (N=32768 -> 4096 rows/core),
weight (1024x1024) replicated; per-core scalar stats are computed
redundantly so no collectives are needed.

Kernel math note: since QB == 1, (x_q @ w_q)*beta*gamma equals
(x @ w_q) * beta * gamma/(gamma+eps) up to the +-(1-eps) clip.  The clip
only affects the row-max element by <=1e-5 relative, and gamma/(gamma+eps)
deviates from 1 by <= eps/gamma ~ 4e-6 -- both far below the bf16 rounding
used for the matmul (~2e-3).  So the kernel never materializes x_q or even
gamma; it feeds bf16(x) to the tensor engine and multiplies the output by
the scalar beta.  The sign itself is computed as (w >= a) - 0.5 = +-0.5 on
the DVE (sign() only exists on ACT, which is slower and busier here); the
missing factor 2 is folded into the output scale: out = (x @ wq') * 2beta.
(Measured end-to-end scale-rel err 3.3e-3 vs 2e-2 gate.)

Design (v5) -- the device kernel is a pure bf16 matmul stream; the
preamble is organized around the measured DMA-queue behavior:
  - x is transposed, cast to bf16 and laid out feature-major on the
    HOST ([128, 8, R]); no PE transposes, no DVE cast on device.
    Output is stored bf16 (halves store traffic); host upcasts.
  - Nothing on the PE can run before the weight mean -> signs, so the
    4MiB weight load IS the critical path.  Measured: one HWDGE ring
    pipelines 512KiB chunks at ~2.2-2.5us each (receipt-bound), SWDGE
    is slower still -- so w rides BOTH HWDGE rings (4 chunks each),
    nothing else touches them, and x waits: chunk 0 on SWDGE now (the
    ramp needs it), the rest data-gated behind the mean via token
    writes into their DMA target slices (engine program order alone
    gets reordered by the scheduler).
  - Per-chunk partial sums ride the arrivals: ACT (accum_out, 1.4us)
    takes the sync-ring chunks, DVE (tensor_reduce, 1.2us) the scalar
    ring's -- two chains that each keep pace with their ring, so the
    mean is ready ~1.4us after the last chunk lands.  The abs-maxes
    (only beta needs them, ~11us later) run on GPSIMD off the path.
  - Signs are 16 half-chunk tensor_scalar ops on DVE (~0.5us each);
    the first THREE tiles' matmuls interleave chunk-major with sign
    production so the PE ramps without stalling (PSUM fits exactly
    3 x 2 banks + 2 scratch banks).
  - 48 warm-up matmuls run under the weight DMA so the HAM clock gate
    is at 8/8 when the real stream starts; the steady stream is 512
    N=512 bf16 matmuls at ~216ns median.
  - Evacuations/stores are per 512-col half, alternating both HWDGE
    rings, shortening the post-stream tail.
"""

import sys

import numpy as np

if "/opt/trn_rl_repo" not in sys.path:
    sys.path.insert(0, "/opt/trn_rl_repo")

N_CORES = 8
N_FEAT = 1024
N_OUT = 1024
P = 128
KC = N_FEAT // P  # 8 contraction chunks of 128
N_WARM = 48  # warm-up matmuls issued under the weight DMA
RAMP = 3  # tiles interleaved during sign production

_NC_CACHE = {}
_PATCHED = False


def _split_multi_waits(nc, max_waits=1):
    """The walrus build in this image rejects instructions carrying more
    than one sync-wait ("Too many sync wait commands").  Tile's semaphore
    assignment attaches one wait per producer proc, so hoist surplus waits
    onto NOP carrier instructions inserted immediately before the waiting
    instruction on the same engine (waits execute before the instruction
    body, so this preserves semantics exactly)."""
    import bass_rust

    for fn in nc.m.functions:
        for blk in fn.blocks:
            insts = blk.instructions  # live list
            i = 0
            while i < len(insts):
                ins = insts[i]
                si = getattr(ins, "sync_info", None)
                if si is None:
                    i += 1
                    continue
                waits = list(si.on_wait)
                if len(waits) <= max_waits:
                    i += 1
                    continue
                keep = waits[:max_waits]
                surplus = waits[max_waits:]
                si.on_wait = keep
                carriers = []
                cur_list = nc.cur_bb.bb.instructions
                for j in range(0, len(surplus), max_waits):
                    nop = nc.engines[ins.engine].nop(nofuse=True)
                    nop.ins.sync_info = bass_rust.SyncInfo(
                        on_wait=surplus[j : j + max_waits], on_update=[]
                    )
                    popped = cur_list.pop()
                    assert popped is nop.ins
                    carriers.append(nop.ins)
                for k, c in enumerate(carriers):
                    insts.insert(i + k, c)
                i += len(carriers) + 1


def _patch_tile_drain():
    global _PATCHED
    if _PATCHED:
        return
    _PATCHED = True
    import concourse.tile as tile

    orig = tile.TileContext._drain_and_barrier

    def patched(self, tick_clock, wait_clock):
        orig(self, tick_clock, wait_clock)
        _split_multi_waits(self.nc)

    tile.TileContext._drain_and_barrier = patched


def _build_nc(rows_per_core: int):
    import concourse.bass as bass
    import concourse.mybir as mybir
    import concourse.tile as tile

    _patch_tile_drain()

    f32 = mybir.dt.float32
    bf16 = mybir.dt.bfloat16
    R = rows_per_core
    assert R % P == 0
    T = R // P

    nc = bass.Bass("TRN2", target_bir_lowering=False, debug=False)
    # xt[p, c*R + r] = x[r, c*128 + p], prepared host-side in bf16
    xt_h = nc.declare_dram_parameter("xt", [P, KC * R], bf16, isOutput=False)
    w_h = nc.declare_dram_parameter("weight", [N_FEAT, N_OUT], f32, isOutput=False)
    o_h = nc.declare_dram_parameter("out", [R, N_OUT], bf16, isOutput=True)
    b_h = nc.declare_dram_parameter("bout", [1, 2], f32, isOutput=True)

    xt_ap = xt_h[:, :].rearrange("p (c r) -> p c r", c=KC)
    o_ap = o_h[:, :]
    # weight[c*128 + p, n] -> [p, c, n]
    w_ap = w_h[:, :].rearrange("(c p) n -> p c n", p=P)

    # x chunk row boundaries: chunk 0 = exactly the RAMP tiles
    # (ungated, smallest possible contention with the weight load),
    # the rest 512-row (1MiB) chunks gated behind the mean
    xb = [0, RAMP * P, RAMP * P + 640]
    while xb[-1] < R:
        xb.append(min(xb[-1] + 512, R))
    n_xch = len(xb) - 1

    with tile.TileContext(nc) as tc:
        with (
            tc.tile_pool(name="wpool", bufs=1) as wpool,
            tc.tile_pool(name="xtpool", bufs=1) as xtpool,
            tc.tile_pool(name="opool", bufs=6) as opool,
            tc.tile_pool(name="pspool", bufs=3, space="PSUM") as pspool,
            tc.tile_pool(name="ps1pool", bufs=2, space="PSUM") as ps1pool,
        ):
            # ---- SBUF-resident tensors ----
            w32 = wpool.tile([P, KC, N_OUT], f32, tag="w32")
            wq = wpool.tile([P, KC, N_OUT], bf16, tag="wq")
            wsum = wpool.tile([P, KC], f32, tag="wsum")
            wmax = wpool.tile([P, KC], f32, tag="wmax")
            ssum = wpool.tile([P, 1], f32, tag="ssum")
            bmax = wpool.tile([P, 1], f32, tag="bmax")
            pack2 = wpool.tile([1, 2], f32, tag="pack2")
            ones128 = wpool.tile([P, P], f32, tag="ones128")
            stats = wpool.tile([P, 2], f32, tag="stats")
            token = wpool.tile([1, 1], bf16, tag="token")
            onesb = wpool.tile([P, 512], bf16, tag="onesb")
            scrap = wpool.tile([P, N_OUT], bf16, tag="scrap")
            xt = xtpool.tile([P, KC, R], bf16, tag="xt")

            nc.vector.memset(pack2, 0.0)
            nc.vector.memset(ones128, 1.0)
            nc.vector.memset(onesb, 1.0)

            # ---- weight DMA: 8 x 512KiB chunks on the two HWDGE rings
            # (even chunks sync, odd chunks scalar); x chunk 0 rides
            # SWDGE concurrently, the rest are gated behind the mean ----
            w_engines = [nc.sync, nc.scalar]
            nc.gpsimd.dma_start(
                out=xt[:, :, 0 : xb[1]], in_=xt_ap[:, :, 0 : xb[1]]
            )
            for c in range(KC):
                w_engines[c % 2].dma_start(out=w32[:, c, :], in_=w_ap[:, c, :])

            # ---- PE warm-up under the weight DMA: keeps the HAM clock
            # gate from parking at 4/8 (1.2GHz) before the real stream
            warm_ps = ps1pool.tile([P, 512], f32, tag="scratch")
            for _ in range(N_WARM):
                nc.tensor.matmul(
                    warm_ps, onesb[:, 0:P], onesb, start=True, stop=True
                )

            # ---- per-chunk sums ride the arrivals: ACT takes the sync
            # ring's chunks (accum_out on a throwaway copy), DVE the
            # scalar ring's, so each chain keeps pace with its ring ----
            for c in range(KC):
                if c % 2 == 0:
                    nc.scalar.activation(
                        out=scrap, in_=w32[:, c, :],
                        func=mybir.ActivationFunctionType.Copy,
                        bias=0.0, scale=1.0,
                        accum_out=wsum[:, c : c + 1],
                    )
                else:
                    nc.vector.tensor_reduce(
                        wsum[:, c : c + 1], w32[:, c, :],
                        axis=mybir.AxisListType.X, op=mybir.AluOpType.add,
                    )
            # mean fast path: one ones[128,128] matmul both reduces
            # across partitions AND replicates the total to all 128
            # output partitions.  This chain gates the signs and
            # therefore every matmul, so it is kept minimal.
            nc.vector.tensor_reduce(
                ssum, wsum, axis=mybir.AxisListType.X, op=mybir.AluOpType.add
            )
            na_ps = ps1pool.tile([P, 1], f32, tag="scratch")
            nc.tensor.matmul(na_ps, ones128, ssum, start=True, stop=True)
            nc.vector.tensor_scalar_mul(
                stats[:, 0:1], na_ps, 1.0 / float(N_FEAT * N_OUT)
            )
            mean_a = stats[:, 0:1]

            # gate the remaining x loads behind the full weight arrival:
            # a token derived from ssum is written INTO each chunk's DMA
            # target slice, so the DMA (same-region write) must follow it
            nc.vector.tensor_copy(out=token, in_=ssum[0:1, 0:1])
            for q in range(1, n_xch):
                nc.vector.tensor_copy(
                    out=xt[0:1, 0:1, xb[q] : xb[q] + 1], in_=token
                )

            # signs on DVE in 512-col halves, chunk-major: wq' =
            # (w >= a) - 0.5 = +-0.5; each half unblocks the matching
            # (c, h) matmuls of the ramp tiles as it lands
            for c in range(KC):
                for h in range(2):
                    nc.vector.tensor_scalar(
                        out=wq[:, c, h * 512 : (h + 1) * 512],
                        in0=w32[:, c, h * 512 : (h + 1) * 512],
                        scalar1=mean_a,
                        scalar2=0.5,
                        op0=mybir.AluOpType.is_ge,
                        op1=mybir.AluOpType.subtract,
                    )

            # the gated x loads; their token writes above are the hard
            # dependency ordering them after the weight arrival
            for q in range(1, n_xch):
                nc.gpsimd.dma_start(
                    out=xt[:, :, xb[q] : xb[q + 1]],
                    in_=xt_ap[:, :, xb[q] : xb[q + 1]],
                )

            # ---- beta: entirely OFF the device critical path.  The
            # output is stored unscaled (bf16 is scale-invariant); beta
            # ships out as a tiny tensor and the HOST folds 2*beta into
            # the f32 upcast it already performs.  DVE computes the
            # abs-maxes after the signs; gpsimd does the cross-partition
            # max and the 8-byte store. ----
            # (token writes into each wmax slice, derived from the last
            # sign half, stop the scheduler hoisting these ahead of the
            # critical ssum -> mean -> signs chain on DVE)
            nc.vector.tensor_copy(
                out=stats[0:1, 1:2], in_=wq[0:1, KC - 1, N_OUT - 1 : N_OUT]
            )
            for c in range(KC):
                nc.vector.tensor_copy(
                    out=wmax[0:1, c : c + 1], in_=stats[0:1, 1:2]
                )
                nc.vector.tensor_reduce(
                    wmax[:, c : c + 1], w32[:, c, :],
                    axis=mybir.AxisListType.X, op=mybir.AluOpType.max,
                    apply_absolute_value=True,
                )
            nc.vector.tensor_reduce(
                bmax, wmax, axis=mybir.AxisListType.X, op=mybir.AluOpType.max
            )
            nc.gpsimd.tensor_reduce(
                pack2[:, 1:2], bmax, axis=mybir.AxisListType.C,
                op=mybir.AluOpType.max,
            )
            nc.gpsimd.dma_start(out=b_h[:, :], in_=pack2)

            def lhs(t, c):
                return xt[:, c, t * P : (t + 1) * P]

            def emit_evac(t, ps):
                # two half evacs + half stores: halves the ACT latency on
                # the tail and balances the store rings
                o = opool.tile([P, N_OUT], bf16, tag="o", name=f"o_{t}")
                for h in range(2):
                    nc.scalar.activation(
                        out=o[:, h * 512 : (h + 1) * 512],
                        in_=ps[:, h * 512 : (h + 1) * 512],
                        func=mybir.ActivationFunctionType.Copy,
                        bias=0.0, scale=1.0,
                    )
                    w_engines[h].dma_start(
                        out=o_ap[t * P : (t + 1) * P, h * 512 : (h + 1) * 512],
                        in_=o[:, h * 512 : (h + 1) * 512],
                    )

            # ---- ramp: tiles 0..RAMP-1 interleaved chunk-major so each
            # arriving sign half feeds RAMP matmuls ----
            assert T >= RAMP
            ramp_ps = [
                pspool.tile([P, N_OUT], f32, tag="ps", name=f"ps_i{t}")
                for t in range(RAMP)
            ]
            for c in range(KC):
                for h in range(2):
                    for t in range(RAMP):
                        nc.tensor.matmul(
                            ramp_ps[t][:, h * 512 : (h + 1) * 512],
                            lhs(t, c),
                            wq[:, c, h * 512 : (h + 1) * 512],
                            start=(c == 0),
                            stop=(c == KC - 1),
                        )
            for t in range(RAMP):
                emit_evac(t, ramp_ps[t])

            # ---- steady stream: everything resident, pure matmuls ----
            for t in range(RAMP, T):
                ps = pspool.tile([P, N_OUT], f32, tag="ps")
                for c in range(KC):
                    for h in range(2):
                        nc.tensor.matmul(
                            ps[:, h * 512 : (h + 1) * 512],
                            lhs(t, c),
                            wq[:, c, h * 512 : (h + 1) * 512],
                            start=(c == 0),
                            stop=(c == KC - 1),
                        )
                emit_evac(t, ps)

    return nc


def _get_nc(rows_per_core: int):
    if rows_per_core not in _NC_CACHE:
        _NC_CACHE[rows_per_core] = _build_nc(rows_per_core)
    return _NC_CACHE[rows_per_core]


def _prep_core_inputs(x, weight):
    """Host-side shard + layout: per-core feature-major bf16 xT."""
    import ml_dtypes

    n = x.shape[0]
    rpc = n // N_CORES
    in_maps = []
    for i in range(N_CORES):
        xi = x[i * rpc : (i + 1) * rpc]
        # xt[p, c, r] = xi[r, c*128 + p]
        xt = xi.reshape(rpc, KC, P).transpose(2, 1, 0)
        xt = np.ascontiguousarray(xt.astype(ml_dtypes.bfloat16))
        xt = xt.reshape(P, KC * rpc)
        in_maps.append({"xt": xt, "weight": weight})
    return in_maps, rpc


def run(x, weight, trace=False, trace_cores=None):
    """Run on 8 cores; returns (out, BassKernelResults)."""
    from concourse.bass_utils import run_bass_kernel_spmd

    x = np.ascontiguousarray(np.asarray(x, dtype=np.float32))
    weight = np.ascontiguousarray(np.asarray(weight, dtype=np.float32))
    n = x.shape[0]
    assert n % N_CORES == 0
    in_maps, rpc = _prep_core_inputs(x, weight)
    nc = _get_nc(rpc)
    kwargs = {}
    if trace:
        kwargs["trace"] = True
        if trace_cores is not None:
            kwargs["trace_cores"] = trace_cores
    res = run_bass_kernel_spmd(nc, in_maps, core_ids=list(range(N_CORES)), **kwargs)
    # signs on device are +-0.5 and the output is stored unscaled, so the
    # final scale is 2*beta, folded into the bf16 -> f32 upcast here
    beta = float(np.asarray(res.results[0]["bout"], dtype=np.float32)[0, 1])
    out = np.concatenate([r["out"] for r in res.results], axis=0)
    out = np.asarray(out, dtype=np.float32) * np.float32(2.0 * beta)
    return out, res


def kernel(x, weight):
    out, _ = run(x, weight)
    return out


# revision 8
# speedup vs baseline: 1.0706x; 1.0706x over previous
"""BitLinear forward on 8 Trainium2 NeuronCores.

out = (x_q @ w_q) * (beta * gamma)
  a      = mean(weight);  w_q = sign(weight - a)
  gamma  = max|x| per row; x_q = clip(x/(gamma+eps), -(1-eps), 1-eps)
  beta   = max|weight|

Sharding: data-parallel over rows of x (N=32768 -> 4096 rows/core),
weight (1024x1024) replicated; per-core scalar stats are computed
redundantly so no collectives are needed.

Kernel math note: since QB == 1, (x_q @ w_q)*beta*gamma equals
(x @ w_q) * beta * gamma/(gamma+eps) up to the +-(1-eps) clip.  The clip
only affects the row-max element by <=1e-5 relative, and gamma/(gamma+eps)
deviates from 1 by <= eps/gamma ~ 4e-6 -- both far below the bf16 rounding
used for the matmul (~2e-3).  So the kernel never materializes x_q or even
gamma; it feeds bf16(x) to the tensor engine and multiplies the output by
the scalar beta.  The sign itself is computed as (w >= a) - 0.5 = +-0.5 on
the DVE (sign() only exists on ACT, which is slower and busier here); the
missing factor 2 is folded into the output scale: out = (x @ wq') * 2beta.
(Measured end-to-end scale-rel err 3.3e-3 vs 2e-2 gate.)

Design (v5) -- the device kernel is a pure bf16 matmul stream; the
preamble is organized around the measured DMA-queue behavior:
  - x is transposed, cast to bf16 and laid out feature-major on the
    HOST ([128, 8, R]); no PE transposes, no DVE cast on device.
    Output is stored bf16 (halves store traffic); host upcasts.
  - Nothing on the PE can run before the weight mean -> signs, so the
    4MiB weight load IS the critical path.  Measured: one HWDGE ring
    pipelines 512KiB chunks at ~2.2-2.5us each (receipt-bound), SWDGE
    is slower still -- so w rides BOTH HWDGE rings (4 chunks each),
    nothing else touches them, and x waits: chunk 0 on SWDGE now (the
    ramp needs it), the rest data-gated behind the mean via token
    writes into their DMA target slices (engine program order alone
    gets reordered by the scheduler).
  - Per-chunk partial sums ride the arrivals: ACT (accum_out, 1.4us)
    takes the sync-ring chunks, DVE (tensor_reduce, 1.2us) the scalar
    ring's -- two chains that each keep pace with their ring, so the
    mean is ready ~1.4us after the last chunk lands.  The abs-maxes
    (only beta needs them, ~11us later) run on GPSIMD off the path.
  - Signs are 16 half-chunk tensor_scalar ops on DVE (~0.5us each);
    the first THREE tiles' matmuls interleave chunk-major with sign
    production so the PE ramps without stalling (PSUM fits exactly
    3 x 2 banks + 2 scratch banks).
  - 48 warm-up matmuls run under the weight DMA so the HAM clock gate
    is at 8/8 when the real stream starts; the steady stream is 512
    N=512 bf16 matmuls at ~216ns median.
  - Evacuations/stores are per 512-col half, alternating both HWDGE
    rings, shortening the post-stream tail.
"""

import sys

import numpy as np

if "/opt/trn_rl_repo" not in sys.path:
    sys.path.insert(0, "/opt/trn_rl_repo")

N_CORES = 8
N_FEAT = 1024
N_OUT = 1024
P = 128
KC = N_FEAT // P  # 8 contraction chunks of 128
N_WARM = 48  # warm-up matmuls issued under the weight DMA
RAMP = 2  # tiles interleaved during sign production

_NC_CACHE = {}
_PATCHED = False


def _split_multi_waits(nc, max_waits=1):
    """The walrus build in this image rejects instructions carrying more
    than one sync-wait ("Too many sync wait commands").  Tile's semaphore
    assignment attaches one wait per producer proc, so hoist surplus waits
    onto NOP carrier instructions inserted immediately before the waiting
    instruction on the same engine (waits execute before the instruction
    body, so this preserves semantics exactly)."""
    import bass_rust

    for fn in nc.m.functions:
        for blk in fn.blocks:
            insts = blk.instructions  # live list
            i = 0
            while i < len(insts):
                ins = insts[i]
                si = getattr(ins, "sync_info", None)
                if si is None:
                    i += 1
                    continue
                waits = list(si.on_wait)
                if len(waits) <= max_waits:
                    i += 1
                    continue
                keep = waits[:max_waits]
                surplus = waits[max_waits:]
                si.on_wait = keep
                carriers = []
                cur_list = nc.cur_bb.bb.instructions
                for j in range(0, len(surplus), max_waits):
                    nop = nc.engines[ins.engine].nop(nofuse=True)
                    nop.ins.sync_info = bass_rust.SyncInfo(
                        on_wait=surplus[j : j + max_waits], on_update=[]
                    )
                    popped = cur_list.pop()
                    assert popped is nop.ins
                    carriers.append(nop.ins)
                for k, c in enumerate(carriers):
                    insts.insert(i + k, c)
                i += len(carriers) + 1


def _patch_tile_drain():
    global _PATCHED
    if _PATCHED:
        return
    _PATCHED = True
    import concourse.tile as tile

    orig = tile.TileContext._drain_and_barrier

    def patched(self, tick_clock, wait_clock):
        orig(self, tick_clock, wait_clock)
        _split_multi_waits(self.nc)

    tile.TileContext._drain_and_barrier = patched


def _build_nc(rows_per_core: int):
    import concourse.bass as bass
    import concourse.mybir as mybir
    import concourse.tile as tile

    _patch_tile_drain()

    f32 = mybir.dt.float32
    bf16 = mybir.dt.bfloat16
    R = rows_per_core
    assert R % P == 0
    T = R // P

    nc = bass.Bass("TRN2", target_bir_lowering=False, debug=False)
    # xt[p, c*R + r] = x[r, c*128 + p], prepared host-side in bf16
    xt_h = nc.declare_dram_parameter("xt", [P, KC * R], bf16, isOutput=False)
    w_h = nc.declare_dram_parameter("weight", [N_FEAT, N_OUT], f32, isOutput=False)
    o_h = nc.declare_dram_parameter("out", [R, N_OUT], bf16, isOutput=True)
    b_h = nc.declare_dram_parameter("bout", [1, 2], f32, isOutput=True)

    xt_ap = xt_h[:, :].rearrange("p (c r) -> p c r", c=KC)
    o_ap = o_h[:, :]
    # weight[c*128 + p, n] -> [p, c, n]
    w_ap = w_h[:, :].rearrange("(c p) n -> p c n", p=P)

    # x chunk row boundaries: chunk 0 = exactly the RAMP tiles
    # (ungated, smallest possible contention with the weight load),
    # the rest 512-row (1MiB) chunks gated behind the mean
    xb = [0, RAMP * P]
    while xb[-1] < R:
        xb.append(min(xb[-1] + 512, R))
    n_xch = len(xb) - 1

    with tile.TileContext(nc) as tc:
        with (
            tc.tile_pool(name="wpool", bufs=1) as wpool,
            tc.tile_pool(name="xtpool", bufs=1) as xtpool,
            tc.tile_pool(name="opool", bufs=6) as opool,
            tc.tile_pool(name="pspool", bufs=3, space="PSUM") as pspool,
            tc.tile_pool(name="ps1pool", bufs=2, space="PSUM") as ps1pool,
        ):
            # ---- SBUF-resident tensors ----
            w32 = wpool.tile([P, KC, N_OUT], f32, tag="w32")
            wq = wpool.tile([P, KC, N_OUT], bf16, tag="wq")
            wsum = wpool.tile([P, KC], f32, tag="wsum")
            wmax = wpool.tile([P, KC], f32, tag="wmax")
            ssum = wpool.tile([P, 1], f32, tag="ssum")
            bmax = wpool.tile([P, 1], f32, tag="bmax")
            pack2 = wpool.tile([1, 2], f32, tag="pack2")
            ones128 = wpool.tile([P, P], f32, tag="ones128")
            stats = wpool.tile([P, 2], f32, tag="stats")
            token = wpool.tile([1, 1], bf16, tag="token")
            onesb = wpool.tile([P, 512], bf16, tag="onesb")
            scrap = wpool.tile([P, N_OUT], bf16, tag="scrap")
            xt = xtpool.tile([P, KC, R], bf16, tag="xt")

            nc.vector.memset(pack2, 0.0)
            nc.vector.memset(ones128, 1.0)
            nc.vector.memset(onesb, 1.0)

            # ---- weight DMA: 8 x 512KiB chunks on the two HWDGE rings
            # (even chunks sync, odd chunks scalar); x chunk 0 rides
            # SWDGE concurrently, the rest are gated behind the mean ----
            w_engines = [nc.sync, nc.scalar]
            nc.gpsimd.dma_start(out=w32[:, 7, :], in_=w_ap[:, 7, :])
            nc.gpsimd.dma_start(
                out=xt[:, :, 0 : xb[1]], in_=xt_ap[:, :, 0 : xb[1]]
            )
            for c in range(KC - 1):
                w_engines[c % 2].dma_start(out=w32[:, c, :], in_=w_ap[:, c, :])

            # ---- PE warm-up under the weight DMA: keeps the HAM clock
            # gate from parking at 4/8 (1.2GHz) before the real stream
            warm_ps = ps1pool.tile([P, 512], f32, tag="scratch")
            for _ in range(N_WARM):
                nc.tensor.matmul(
                    warm_ps, onesb[:, 0:P], onesb, start=True, stop=True
                )

            # ---- per-chunk sums ride the arrivals: ACT takes the sync
            # ring's chunks (accum_out on a throwaway copy), DVE the
            # scalar ring's, so each chain keeps pace with its ring ----
            for c in (0, 7, 2, 4, 6):
                nc.vector.tensor_reduce(
                    wsum[:, c : c + 1], w32[:, c, :],
                    axis=mybir.AxisListType.X, op=mybir.AluOpType.add,
                )
            for c in (1, 3, 5):
                nc.scalar.activation(
                    out=scrap, in_=w32[:, c, :],
                    func=mybir.ActivationFunctionType.Copy,
                    bias=0.0, scale=1.0,
                    accum_out=wsum[:, c : c + 1],
                )
            # mean fast path: one ones[128,128] matmul both reduces
            # across partitions AND replicates the total to all 128
            # output partitions.  This chain gates the signs and
            # therefore every matmul, so it is kept minimal.
            nc.vector.tensor_reduce(
                ssum, wsum, axis=mybir.AxisListType.X, op=mybir.AluOpType.add
            )
            na_ps = ps1pool.tile([P, 1], f32, tag="scratch")
            nc.tensor.matmul(na_ps, ones128, ssum, start=True, stop=True)
            nc.vector.tensor_scalar_mul(
                stats[:, 0:1], na_ps, 1.0 / float(N_FEAT * N_OUT)
            )
            mean_a = stats[:, 0:1]

            # gate the remaining x loads behind the full weight arrival:
            # a token derived from ssum is written INTO each chunk's DMA
            # target slice, so the DMA (same-region write) must follow it
            nc.vector.tensor_copy(out=token, in_=ssum[0:1, 0:1])
            for q in range(1, n_xch):
                nc.vector.tensor_copy(
                    out=xt[0:1, 0:1, xb[q] : xb[q] + 1], in_=token
                )

            # signs on DVE in 512-col halves, chunk-major: wq' =
            # (w >= a) - 0.5 = +-0.5; each half unblocks the matching
            # (c, h) matmuls of the ramp tiles as it lands
            for c in range(KC):
                for h in range(2):
                    nc.vector.tensor_scalar(
                        out=wq[:, c, h * 512 : (h + 1) * 512],
                        in0=w32[:, c, h * 512 : (h + 1) * 512],
                        scalar1=mean_a,
                        scalar2=0.5,
                        op0=mybir.AluOpType.is_ge,
                        op1=mybir.AluOpType.subtract,
                    )

            # the gated x loads; their token writes above are the hard
            # dependency ordering them after the weight arrival
            for q in range(1, n_xch):
                nc.gpsimd.dma_start(
                    out=xt[:, :, xb[q] : xb[q + 1]],
                    in_=xt_ap[:, :, xb[q] : xb[q + 1]],
                )

            # ---- beta: entirely OFF the device critical path.  The
            # output is stored unscaled (bf16 is scale-invariant); beta
            # ships out as a tiny tensor and the HOST folds 2*beta into
            # the f32 upcast it already performs.  DVE computes the
            # abs-maxes after the signs; gpsimd does the cross-partition
            # max and the 8-byte store. ----
            # (token writes into each wmax slice, derived from the last
            # sign half, stop the scheduler hoisting these ahead of the
            # critical ssum -> mean -> signs chain on DVE)
            nc.vector.tensor_copy(
                out=stats[0:1, 1:2], in_=wq[0:1, KC - 1, N_OUT - 1 : N_OUT]
            )
            for c in range(KC):
                nc.vector.tensor_copy(
                    out=wmax[0:1, c : c + 1], in_=stats[0:1, 1:2]
                )
                nc.vector.tensor_reduce(
                    wmax[:, c : c + 1], w32[:, c, :],
                    axis=mybir.AxisListType.X, op=mybir.AluOpType.max,
                    apply_absolute_value=True,
                )
            nc.vector.tensor_reduce(
                bmax, wmax, axis=mybir.AxisListType.X, op=mybir.AluOpType.max
            )
            nc.gpsimd.tensor_reduce(
                pack2[:, 1:2], bmax, axis=mybir.AxisListType.C,
                op=mybir.AluOpType.max,
            )
            nc.gpsimd.dma_start(out=b_h[:, :], in_=pack2)

            def lhs(t, c):
                return xt[:, c, t * P : (t + 1) * P]

            def emit_evac(t, ps):
                # two half evacs + half stores: halves the ACT latency on
                # the tail and balances the store rings
                o = opool.tile([P, N_OUT], bf16, tag="o", name=f"o_{t}")
                for h in range(2):
                    nc.scalar.activation(
                        out=o[:, h * 512 : (h + 1) * 512],
                        in_=ps[:, h * 512 : (h + 1) * 512],
                        func=mybir.ActivationFunctionType.Copy,
                        bias=0.0, scale=1.0,
                    )
                    w_engines[h].dma_start(
                        out=o_ap[t * P : (t + 1) * P, h * 512 : (h + 1) * 512],
                        in_=o[:, h * 512 : (h + 1) * 512],
                    )

            # ---- ramp: tiles 0..RAMP-1 interleaved chunk-major so each
            # arriving sign half feeds RAMP matmuls ----
            assert T >= RAMP
            ramp_ps = [
                pspool.tile([P, N_OUT], f32, tag="ps", name=f"ps_i{t}")
                for t in range(RAMP)
            ]
            for c in range(KC):
                for h in range(2):
                    for t in range(RAMP):
                        nc.tensor.matmul(
                            ramp_ps[t][:, h * 512 : (h + 1) * 512],
                            lhs(t, c),
                            wq[:, c, h * 512 : (h + 1) * 512],
                            start=(c == 0),
                            stop=(c == KC - 1),
                        )
            for t in range(RAMP):
                emit_evac(t, ramp_ps[t])

            # ---- steady stream: everything resident, pure matmuls ----
            for t in range(RAMP, T):
                ps = pspool.tile([P, N_OUT], f32, tag="ps")
                for c in range(KC):
                    for h in range(2):
                        nc.tensor.matmul(
                            ps[:, h * 512 : (h + 1) * 512],
                            lhs(t, c),
                            wq[:, c, h * 512 : (h + 1) * 512],
                            start=(c == 0),
                            stop=(c == KC - 1),
                        )
                emit_evac(t, ps)

    return nc


def _get_nc(rows_per_core: int):
    if rows_per_core not in _NC_CACHE:
        _NC_CACHE[rows_per_core] = _build_nc(rows_per_core)
    return _NC_CACHE[rows_per_core]


def _prep_core_inputs(x, weight):
    """Host-side shard + layout: per-core feature-major bf16 xT."""
    import ml_dtypes

    n = x.shape[0]
    rpc = n // N_CORES
    in_maps = []
    for i in range(N_CORES):
        xi = x[i * rpc : (i + 1) * rpc]
        # xt[p, c, r] = xi[r, c*128 + p]
        xt = xi.reshape(rpc, KC, P).transpose(2, 1, 0)
        xt = np.ascontiguousarray(xt.astype(ml_dtypes.bfloat16))
        xt = xt.reshape(P, KC * rpc)
        in_maps.append({"xt": xt, "weight": weight})
    return in_maps, rpc


def run(x, weight, trace=False, trace_cores=None):
    """Run on 8 cores; returns (out, BassKernelResults)."""
    from concourse.bass_utils import run_bass_kernel_spmd

    x = np.ascontiguousarray(np.asarray(x, dtype=np.float32))
    weight = np.ascontiguousarray(np.asarray(weight, dtype=np.float32))
    n = x.shape[0]
    assert n % N_CORES == 0
    in_maps, rpc = _prep_core_inputs(x, weight)
    nc = _get_nc(rpc)
    kwargs = {}
    if trace:
        kwargs["trace"] = True
        if trace_cores is not None:
            kwargs["trace_cores"] = trace_cores
    res = run_bass_kernel_spmd(nc, in_maps, core_ids=list(range(N_CORES)), **kwargs)
    # signs on device are +-0.5 and the output is stored unscaled, so the
    # final scale is 2*beta, folded into the bf16 -> f32 upcast here
    beta = float(np.asarray(res.results[0]["bout"], dtype=np.float32)[0, 1])
    out = np.concatenate([r["out"] for r in res.results], axis=0)
    out = np.asarray(out, dtype=np.float32) * np.float32(2.0 * beta)
    return out, res


def kernel(x, weight):
    out, _ = run(x, weight)
    return out


# revision 9
# speedup vs baseline: 1.1023x; 1.0296x over previous
"""BitLinear forward on 8 Trainium2 NeuronCores.

out = (x_q @ w_q) * (beta * gamma)
  a      = mean(weight);  w_q = sign(weight - a)
  gamma  = max|x| per row; x_q = clip(x/(gamma+eps), -(1-eps), 1-eps)
  beta   = max|weight|

Sharding: data-parallel over rows of x (N=32768 -> 4096 rows/core),
weight (1024x1024) replicated; per-core scalar stats are computed
redundantly so no collectives are needed.

Kernel math notes:
  - Since QB == 1, (x_q @ w_q)*beta*gamma equals (x @ w_q)*beta *
    gamma/(gamma+eps) up to the +-(1-eps) clip; the clip and eps terms
    are < 1e-5 relative, far below the 16-bit rounding of the matmul.
    So the kernel computes (x @ w_q) scaled by beta; gamma is never
    materialized.
  - The sign is computed as (w >= a) - 0.5 = +-0.5 on the DVE (one
    two-op tensor_scalar); the missing factor 2 rides the final scale.
  - The output is stored UNSCALED in bf16 (bf16 is scale-invariant);
    beta ships out as a tiny [1,2] tensor and the host folds 2*beta
    into the bf16 -> f32 upcast it already performs.  This keeps the
    beta reduction entirely off the device critical path.
  - Features 0..FP8C*128 run as fp8-e4m3 DoubleRow matmuls (2 virtual
    k-rows per cycle, ~2x PE rate); the rest stay bf16.  Measured
    end-to-end scale-rel err 1.25e-2 vs the 2e-2 gate (bf16-only is
    3.3e-3; full fp8 would be 2.5e-2 and fails).

Schedule (per core; times approximate, from perfetto traces):
  - The weight mean gates the signs and therefore every matmul, so the
    4MiB weight load owns the HBM bandwidth: it arrives as 4 x 1MiB
    DMAs, two per HWDGE ring (a ring retires chunk DMAs no faster than
    ~2.2us each regardless of size, so 1MiB is the efficiency knee).
    x chunk 0 (the 2 ramp tiles) rides SWDGE concurrently; all other x
    chunks are data-gated behind the mean via token writes into their
    own DMA target slices (engine program order alone gets reordered
    by the Tile scheduler).
  - Per-chunk partial sums ride the group arrivals, split DVE
    (tensor_reduce, 1.2us) / ACT (accum_out copy, 1.4us).  The
    abs-maxes for beta run on DVE after the signs, token-gated so the
    scheduler cannot hoist them into the critical chain.
  - Signs are 16 half-chunk tensor_scalar ops on DVE (~0.47us each),
    h-major so the fp8 pair (which the DoubleRow matmul consumes
    first) lands first; the first TWO tiles' matmuls interleave with
    sign production (2 matmuls per arriving half = exactly the DVE
    production rate).
  - 48 warm-up matmuls run under the weight DMA so the HAM clock gate
    is at 8/8 when the real stream starts.
  - Steady state: 32 tiles x (2 DoubleRow + 12 bf16) matmuls, N=512,
    ~216ns each; ACT evacuates h0 and DVE h1 of each PSUM tile in
    parallel; half stores alternate the two HWDGE rings.
"""

import sys

import numpy as np

if "/opt/trn_rl_repo" not in sys.path:
    sys.path.insert(0, "/opt/trn_rl_repo")

N_CORES = 8
N_FEAT = 1024
N_OUT = 1024
P = 128
KC = N_FEAT // P  # 8 contraction chunks of 128
FP8C = 2  # leading chunks that run as fp8 DoubleRow (must be even)
KCB = KC - FP8C  # bf16 chunks
N_WARM = 48  # warm-up matmuls issued under the weight DMA
RAMP = 2  # tiles interleaved during sign production

_NC_CACHE = {}
_PATCHED = False


def _split_multi_waits(nc, max_waits=1):
    """The walrus build in this image rejects instructions carrying more
    than one sync-wait ("Too many sync wait commands").  Tile's semaphore
    assignment attaches one wait per producer proc, so hoist surplus waits
    onto NOP carrier instructions inserted immediately before the waiting
    instruction on the same engine (waits execute before the instruction
    body, so this preserves semantics exactly)."""
    import bass_rust

    for fn in nc.m.functions:
        for blk in fn.blocks:
            insts = blk.instructions  # live list
            i = 0
            while i < len(insts):
                ins = insts[i]
                si = getattr(ins, "sync_info", None)
                if si is None:
                    i += 1
                    continue
                waits = list(si.on_wait)
                if len(waits) <= max_waits:
                    i += 1
                    continue
                keep = waits[:max_waits]
                surplus = waits[max_waits:]
                si.on_wait = keep
                carriers = []
                cur_list = nc.cur_bb.bb.instructions
                for j in range(0, len(surplus), max_waits):
                    nop = nc.engines[ins.engine].nop(nofuse=True)
                    nop.ins.sync_info = bass_rust.SyncInfo(
                        on_wait=surplus[j : j + max_waits], on_update=[]
                    )
                    popped = cur_list.pop()
                    assert popped is nop.ins
                    carriers.append(nop.ins)
                for k, c in enumerate(carriers):
                    insts.insert(i + k, c)
                i += len(carriers) + 1


def _patch_tile_drain():
    global _PATCHED
    if _PATCHED:
        return
    _PATCHED = True
    import concourse.tile as tile

    orig = tile.TileContext._drain_and_barrier

    def patched(self, tick_clock, wait_clock):
        orig(self, tick_clock, wait_clock)
        _split_multi_waits(self.nc)

    tile.TileContext._drain_and_barrier = patched


def _build_nc(rows_per_core: int):
    import concourse.bass as bass
    import concourse.mybir as mybir
    import concourse.tile as tile

    _patch_tile_drain()

    f32 = mybir.dt.float32
    bf16 = mybir.dt.bfloat16
    fp8 = mybir.dt.float8e4
    DR = mybir.MatmulPerfMode.DoubleRow
    R = rows_per_core
    assert R % P == 0
    T = R // P

    nc = bass.Bass("TRN2", target_bir_lowering=False, debug=False)
    # xt8[p, c*R + r] = x[r, c*128 + p]          for c in [0, FP8C)
    # xt [p, c*R + r] = x[r, (FP8C+c)*128 + p]   for c in [0, KCB)
    x8_h = nc.declare_dram_parameter("xt8", [P, FP8C * R], fp8, isOutput=False)
    xt_h = nc.declare_dram_parameter("xt", [P, KCB * R], bf16, isOutput=False)
    w_h = nc.declare_dram_parameter("weight", [N_FEAT, N_OUT], f32, isOutput=False)
    o_h = nc.declare_dram_parameter("out", [R, N_OUT], bf16, isOutput=True)
    b_h = nc.declare_dram_parameter("bout", [1, 2], f32, isOutput=True)

    x8_ap = x8_h[:, :].rearrange("p (c r) -> p c r", c=FP8C)
    xt_ap = xt_h[:, :].rearrange("p (c r) -> p c r", c=KCB)
    o_ap = o_h[:, :]
    # weight[c*128 + p, n] -> [p, c, n]
    w_ap = w_h[:, :].rearrange("(c p) n -> p c n", p=P)

    # x chunk row boundaries: chunk 0 = the RAMP tiles (ungated), rest
    # 512-row chunks gated behind the mean
    xb = [0, RAMP * P]
    while xb[-1] < R:
        xb.append(min(xb[-1] + 512, R))
    n_xch = len(xb) - 1

    # DVE/ACT split of the 8 per-chunk sums (by group arrival order)
    DVE_SUM = (0, 1, 4, 6)
    ACT_SUM = (2, 3, 5, 7)

    with tile.TileContext(nc) as tc:
        with (
            tc.tile_pool(name="wpool", bufs=1) as wpool,
            tc.tile_pool(name="xtpool", bufs=1) as xtpool,
            tc.tile_pool(name="opool", bufs=6) as opool,
            tc.tile_pool(name="pspool", bufs=3, space="PSUM") as pspool,
            tc.tile_pool(name="ps1pool", bufs=2, space="PSUM") as ps1pool,
        ):
            # ---- SBUF-resident tensors ----
            w32 = wpool.tile([P, KC, N_OUT], f32, tag="w32")
            wq8 = wpool.tile([P, FP8C, N_OUT], fp8, tag="wq8")
            wq = wpool.tile([P, KCB, N_OUT], bf16, tag="wq")
            wsum = wpool.tile([P, KC], f32, tag="wsum")
            wmax = wpool.tile([P, KC], f32, tag="wmax")
            ssum = wpool.tile([P, 1], f32, tag="ssum")
            bmax = wpool.tile([P, 1], f32, tag="bmax")
            pack2 = wpool.tile([1, 2], f32, tag="pack2")
            ones128 = wpool.tile([P, P], f32, tag="ones128")
            stats = wpool.tile([P, 2], f32, tag="stats")
            token = wpool.tile([1, 1], bf16, tag="token")
            tok8 = wpool.tile([1, 1], fp8, tag="tok8")
            onesb = wpool.tile([P, 512], bf16, tag="onesb")
            scrap = wpool.tile([P, N_OUT], bf16, tag="scrap")
            x8 = xtpool.tile([P, FP8C, R], fp8, tag="x8")
            xt = xtpool.tile([P, KCB, R], bf16, tag="xt")

            nc.vector.memset(pack2, 0.0)
            nc.vector.memset(ones128, 1.0)
            nc.vector.memset(onesb, 1.0)

            # ---- weight DMA: 4 x 1MiB (2 chunks each), two per HWDGE
            # ring; x chunk 0 rides SWDGE concurrently ----
            w_engines = [nc.sync, nc.scalar]
            for g in range(4):
                w_engines[g % 2].dma_start(
                    out=w32[:, 2 * g : 2 * g + 2, :],
                    in_=w_ap[:, 2 * g : 2 * g + 2, :],
                )
            nc.gpsimd.dma_start(
                out=x8[:, :, 0 : xb[1]], in_=x8_ap[:, :, 0 : xb[1]]
            )
            nc.gpsimd.dma_start(
                out=xt[:, :, 0 : xb[1]], in_=xt_ap[:, :, 0 : xb[1]]
            )

            # ---- PE warm-up under the weight DMA: keeps the HAM clock
            # gate from parking at 4/8 (1.2GHz) before the real stream
            warm_ps = ps1pool.tile([P, 512], f32, tag="scratch")
            for _ in range(N_WARM):
                nc.tensor.matmul(
                    warm_ps, onesb[:, 0:P], onesb, start=True, stop=True
                )

            # ---- per-chunk sums ride the group arrivals ----
            for c in DVE_SUM:
                nc.vector.tensor_reduce(
                    wsum[:, c : c + 1], w32[:, c, :],
                    axis=mybir.AxisListType.X, op=mybir.AluOpType.add,
                )
            for c in ACT_SUM:
                nc.scalar.activation(
                    out=scrap, in_=w32[:, c, :],
                    func=mybir.ActivationFunctionType.Copy,
                    bias=0.0, scale=1.0,
                    accum_out=wsum[:, c : c + 1],
                )
            # mean fast path: one ones[128,128] matmul both reduces
            # across partitions AND replicates the total to all 128
            # output partitions
            nc.vector.tensor_reduce(
                ssum, wsum, axis=mybir.AxisListType.X, op=mybir.AluOpType.add
            )
            na_ps = ps1pool.tile([P, 1], f32, tag="scratch")
            nc.tensor.matmul(na_ps, ones128, ssum, start=True, stop=True)
            nc.vector.tensor_scalar_mul(
                stats[:, 0:1], na_ps, 1.0 / float(N_FEAT * N_OUT)
            )
            mean_a = stats[:, 0:1]

            # gate the remaining x loads behind the full weight arrival:
            # tokens derived from ssum are written INTO each chunk's DMA
            # target slices, a WAW dependency the scheduler must honor
            nc.vector.tensor_copy(out=token, in_=ssum[0:1, 0:1])
            nc.vector.tensor_copy(out=tok8, in_=ssum[0:1, 0:1])
            for q in range(1, n_xch):
                nc.vector.tensor_copy(
                    out=x8[0:1, 0:1, xb[q] : xb[q] + 1], in_=tok8
                )
                nc.vector.tensor_copy(
                    out=xt[0:1, 0:1, xb[q] : xb[q] + 1], in_=token
                )

            # signs on DVE in 512-col halves: wq' = (w >= a) - 0.5 =
            # +-0.5 (exact in fp8/bf16); h-major so the fp8 pair that
            # the DoubleRow matmuls consume first lands first
            for h in range(2):
                hs = slice(h * 512, (h + 1) * 512)
                for c in range(KC):
                    dst = wq8[:, c, hs] if c < FP8C else wq[:, c - FP8C, hs]
                    nc.vector.tensor_scalar(
                        out=dst,
                        in0=w32[:, c, hs],
                        scalar1=mean_a,
                        scalar2=0.5,
                        op0=mybir.AluOpType.is_ge,
                        op1=mybir.AluOpType.subtract,
                    )

            # the gated x loads (both dtypes per row chunk)
            for q in range(1, n_xch):
                nc.gpsimd.dma_start(
                    out=x8[:, :, xb[q] : xb[q + 1]],
                    in_=x8_ap[:, :, xb[q] : xb[q + 1]],
                )
                nc.gpsimd.dma_start(
                    out=xt[:, :, xb[q] : xb[q + 1]],
                    in_=xt_ap[:, :, xb[q] : xb[q + 1]],
                )

            # ---- beta: entirely OFF the device critical path.  The
            # output is stored unscaled; beta ships out as a tiny tensor
            # and the HOST folds 2*beta into its f32 upcast.  Token
            # writes stop the scheduler hoisting these DVE reduces into
            # the critical chain above. ----
            nc.vector.tensor_copy(
                out=stats[0:1, 1:2], in_=wq[0:1, KCB - 1, N_OUT - 1 : N_OUT]
            )
            for c in range(KC):
                nc.vector.tensor_copy(
                    out=wmax[0:1, c : c + 1], in_=stats[0:1, 1:2]
                )
                nc.vector.tensor_reduce(
                    wmax[:, c : c + 1], w32[:, c, :],
                    axis=mybir.AxisListType.X, op=mybir.AluOpType.max,
                    apply_absolute_value=True,
                )
            nc.vector.tensor_reduce(
                bmax, wmax, axis=mybir.AxisListType.X, op=mybir.AluOpType.max
            )
            nc.gpsimd.tensor_reduce(
                pack2[:, 1:2], bmax, axis=mybir.AxisListType.C,
                op=mybir.AluOpType.max,
            )
            nc.gpsimd.dma_start(out=b_h[:, :], in_=pack2)

            def emit_tile_mms(groups):
                """groups: list of (psum_tile, t) pairs emitted
                interleaved per (h, k-group) so sign production feeds
                len(groups) matmuls per arriving half."""
                for h in range(2):
                    hs = slice(h * 512, (h + 1) * 512)
                    for gi in range(FP8C // 2 + KCB):
                        for ps, t in groups:
                            if gi < FP8C // 2:
                                nc.tensor.matmul(
                                    ps[:, hs],
                                    x8[
                                        :,
                                        2 * gi : 2 * gi + 2,
                                        t * P : (t + 1) * P,
                                    ],
                                    wq8[:, 2 * gi : 2 * gi + 2, hs],
                                    start=(gi == 0),
                                    stop=False,
                                    perf_mode=DR,
                                )
                            else:
                                cc = gi - FP8C // 2
                                nc.tensor.matmul(
                                    ps[:, hs],
                                    xt[:, cc, t * P : (t + 1) * P],
                                    wq[:, cc, hs],
                                    start=False,
                                    stop=(cc == KCB - 1),
                                )

            def emit_evac(t, ps):
                # ACT evacuates h0, DVE h1 in parallel (different PSUM
                # banks); half stores alternate the two HWDGE rings
                o = opool.tile([P, N_OUT], bf16, tag="o", name=f"o_{t}")
                nc.scalar.activation(
                    out=o[:, 0:512], in_=ps[:, 0:512],
                    func=mybir.ActivationFunctionType.Copy,
                    bias=0.0, scale=1.0,
                )
                nc.sync.dma_start(
                    out=o_ap[t * P : (t + 1) * P, 0:512], in_=o[:, 0:512]
                )
                nc.vector.tensor_copy(out=o[:, 512:1024], in_=ps[:, 512:1024])
                nc.scalar.dma_start(
                    out=o_ap[t * P : (t + 1) * P, 512:1024], in_=o[:, 512:1024]
                )

            # ---- ramp: first RAMP tiles interleaved with sign
            # production, then the steady stream ----
            assert T >= RAMP
            ramp_ps = [
                pspool.tile([P, N_OUT], f32, tag="ps", name=f"ps_i{t}")
                for t in range(RAMP)
            ]
            emit_tile_mms([(ramp_ps[t], t) for t in range(RAMP)])
            for t in range(RAMP):
                emit_evac(t, ramp_ps[t])

            for t in range(RAMP, T):
                ps = pspool.tile([P, N_OUT], f32, tag="ps")
                emit_tile_mms([(ps, t)])
                emit_evac(t, ps)

    return nc


def _get_nc(rows_per_core: int):
    if rows_per_core not in _NC_CACHE:
        _NC_CACHE[rows_per_core] = _build_nc(rows_per_core)
    return _NC_CACHE[rows_per_core]


def _prep_core_inputs(x, weight):
    """Host-side shard + layout: per-core feature-major xT, fp8 for the
    leading FP8C*128 features, bf16 for the rest."""
    import ml_dtypes

    n = x.shape[0]
    rpc = n // N_CORES
    kf = FP8C * P
    in_maps = []
    for i in range(N_CORES):
        xi = x[i * rpc : (i + 1) * rpc]
        x8 = xi[:, :kf].reshape(rpc, FP8C, P).transpose(2, 1, 0)
        x8 = np.ascontiguousarray(x8.astype(ml_dtypes.float8_e4m3fn))
        xt = xi[:, kf:].reshape(rpc, KCB, P).transpose(2, 1, 0)
        xt = np.ascontiguousarray(xt.astype(ml_dtypes.bfloat16))
        in_maps.append(
            {
                "xt8": x8.reshape(P, FP8C * rpc),
                "xt": xt.reshape(P, KCB * rpc),
                "weight": weight,
            }
        )
    return in_maps, rpc


def run(x, weight, trace=False, trace_cores=None):
    """Run on 8 cores; returns (out, BassKernelResults)."""
    from concourse.bass_utils import run_bass_kernel_spmd

    x = np.ascontiguousarray(np.asarray(x, dtype=np.float32))
    weight = np.ascontiguousarray(np.asarray(weight, dtype=np.float32))
    n = x.shape[0]
    assert n % N_CORES == 0
    in_maps, rpc = _prep_core_inputs(x, weight)
    nc = _get_nc(rpc)
    kwargs = {}
    if trace:
        kwargs["trace"] = True
        if trace_cores is not None:
            kwargs["trace_cores"] = trace_cores
    res = run_bass_kernel_spmd(nc, in_maps, core_ids=list(range(N_CORES)), **kwargs)
    # signs on device are +-0.5 and the output is stored unscaled, so
    # the final scale is 2*beta, folded into the bf16 -> f32 upcast
    beta = float(np.asarray(res.results[0]["bout"], dtype=np.float32)[0, 1])
    out = np.concatenate([r["out"] for r in res.results], axis=0)
    out = np.asarray(out, dtype=np.float32) * np.float32(2.0 * beta)
    return out, res


def kernel(x, weight):
    out, _ = run(x, weight)
    return out
